# revision 1
# baseline (speedup 1.0000x reference)
"""GCN block (GCNConv + GraphNorm + ReLU + max-pool) on 8 trn2 cores.

Two-launch design (all-static NEFFs, shared across cores):
  pass1: per-core edge one-hot (modular 16x16) -> deg -> dis = 1/sqrt(deg);
         Y_slice = dis * x_slice.  Host glues dis/Y across cores.
  pass2: dma_gather Y[src] per 128-edge tile, one-hot S=(iota==dstcol)*ew,
         PSUM-accumulated matmul per 256-col window -> agg.T; agg *= dis[dst];
         h.T = W.T @ agg.T; GraphNorm via raw moments per fixed 1792-col graph
         block; ReLU; masked max-pool; PE-transpose h_emb out.
"""
import numpy as np

import concourse.bass as bass
import concourse.bacc as bacc
import concourse.tile as tile
from concourse import mybir
from concourse.bass_utils import run_bass_kernel_spmd
from concourse.masks import make_identity

P = 128
FDIM = 128
EPS = 1e-5
NCORES = 8
CHUNK = 32768
NCHUNK = 4
WINW = 256  # window width (PSUM free dim)

f32 = mybir.dt.float32
f32r = mybir.dt.float32r
i32 = mybir.dt.int32
i16 = mybir.dt.int16
Alu = mybir.AluOpType
Act = mybir.ActivationFunctionType


def _segmented_ranks(sorted_keys):
    """rank of each element within its (contiguous) key segment."""
    n = len(sorted_keys)
    if n == 0:
        return np.zeros(0, dtype=np.int64)
    starts = np.flatnonzero(np.concatenate([[True], sorted_keys[1:] != sorted_keys[:-1]]))
    seg_id = np.cumsum(np.concatenate([[False], sorted_keys[1:] != sorted_keys[:-1]])).astype(np.int64)
    return np.arange(n, dtype=np.int64) - starts[seg_id]


def plan(inputs, edge_index, batch, edge_weight):
    """Host-side sharding/bucketing. Returns dict of static dims + arrays."""
    N = inputs.shape[0]
    G = 64
    cnt = np.bincount(batch, minlength=G).astype(np.int64)
    gstart = np.zeros(G + 1, dtype=np.int64)
    np.cumsum(cnt, out=gstart[1:])
    GPC = G // NCORES
    n0 = gstart[np.arange(NCORES) * GPC]
    n1 = gstart[np.arange(NCORES) * GPC + GPC]
    nloc1 = n1 - n0
    NLOC1 = int(np.ceil(max(1, nloc1.max()) / WINW) * WINW)
    WIN1 = NLOC1 // WINW
    WPG = max(1, int(np.ceil(max(1, cnt.max()) / WINW)))
    GSPAN = WPG * WINW
    WIN2 = GPC * WPG
    NLOC2 = GPC * GSPAN

    src = edge_index[0].astype(np.int64)
    dst = edge_index[1].astype(np.int64)
    E = src.shape[0]
    src_f = np.concatenate([src, np.arange(N, dtype=np.int64)])
    dst_f = np.concatenate([dst, np.arange(N, dtype=np.int64)])
    ew_f = np.concatenate([edge_weight.astype(np.float32), np.ones(N, np.float32)])

    gid = batch.astype(np.int64)[dst_f]
    core_e = gid // GPC
    slot_e = gid % GPC
    col1 = dst_f - n0[core_e]
    w1 = col1 >> 8
    col2 = slot_e * GSPAN + (dst_f - gstart[gid])
    w2 = col2 >> 8
    NCH = max(1, int(np.ceil(N / CHUNK)))
    chunk_e = np.minimum(src_f // CHUNK, NCH - 1)

    # ---------- pass 1 ----------
    key1 = core_e * WIN1 + w1
    o1 = np.argsort(key1, kind="stable")
    k1s = key1[o1]
    r1 = _segmented_ranks(k1s)
    bc1 = np.bincount(key1, minlength=NCORES * WIN1)
    K1 = max(1, int(np.ceil(bc1.max() / P)))
    NT1 = WIN1 * K1

    dsthi1 = np.zeros((NCORES, P, NT1), np.float32)
    dstlo1 = np.zeros((NCORES, P, NT1), np.float32)
    ew1 = np.zeros((NCORES, P, NT1), np.float32)
    core_s = k1s // WIN1
    win_s = k1s % WIN1
    tl = win_s * K1 + r1 // P
    pp = r1 % P
    c1s = col1[o1]
    dsthi1[core_s, pp, tl] = ((c1s >> 4) & 15).astype(np.float32)
    dstlo1[core_s, pp, tl] = (c1s & 15).astype(np.float32)
    ew1[core_s, pp, tl] = ew_f[o1]

    # ---------- pass 2 ----------
    key2 = (core_e * WIN2 + w2) * NCH + chunk_e
    o2 = np.argsort(key2, kind="stable")
    k2s = key2[o2]
    r2 = _segmented_ranks(k2s)
    bc2 = np.bincount(key2, minlength=NCORES * WIN2 * NCH).reshape(NCORES, WIN2, NCH)
    K2C = np.maximum(1, np.ceil(bc2.max(axis=(0, 1)) / P).astype(np.int64))
    K2B = np.zeros(NCH + 1, np.int64)
    np.cumsum(K2C, out=K2B[1:])
    K2S = int(K2B[-1])
    NT2 = WIN2 * K2S

    dstcol2 = np.zeros((NCORES, P, NT2), np.float32)
    ew2 = np.zeros((NCORES, P, NT2), np.float32)
    idx2 = np.zeros((NCORES, 16, NT2 * 8), np.int16)
    core_s = k2s // (WIN2 * NCH)
    win_s = (k2s // NCH) % WIN2
    ch_s = k2s % NCH
    tile_in_chunk = r2 // P
    pp = r2 % P
    tl = win_s * K2S + K2B[ch_s] + tile_in_chunk
    dstcol2[core_s, pp, tl] = (col2[o2] & 255).astype(np.float32)
    ew2[core_s, pp, tl] = ew_f[o2]
    # idx layout per call (w, ch): token t = tile_in_chunk*128 + p at
    # [t % 16, calloff*8 + t // 16] where calloff = w*K2S + K2B[ch]
    tok = tile_in_chunk * P + pp
    srcrel = (src_f[o2] - ch_s * CHUNK).astype(np.int16)
    idx2[core_s, tok % 16, (win_s * K2S + K2B[ch_s]) * 8 + tok // 16] = srcrel
    idx2 = np.broadcast_to(idx2[:, None, :, :], (NCORES, 8, 16, NT2 * 8)).reshape(
        NCORES, P, NT2 * 8
    ).copy()

    # per-core masks / counts
    mask2 = np.zeros((NCORES, 1, NLOC2), np.float32)
    cntinv = np.ones((NCORES, P, GPC), np.float32)
    for c in range(NCORES):
        for s in range(GPC):
            g = c * GPC + s
            mask2[c, 0, s * GSPAN: s * GSPAN + cnt[g]] = 1.0
            cntinv[c, :, s] = 1.0 / max(1, cnt[g])

    x_slices = np.zeros((NCORES, NLOC1, FDIM), np.float32)
    for c in range(NCORES):
        x_slices[c, : nloc1[c]] = inputs[n0[c]: n1[c]]

    dims = dict(N=N, G=G, GPC=GPC, WPG=WPG, GSPAN=GSPAN, WIN1=WIN1, WIN2=WIN2,
                NLOC1=NLOC1, NLOC2=NLOC2, K1=K1, K2C=tuple(int(k) for k in K2C),
                NT1=NT1, NT2=NT2, NCH=NCH)
    chunk_rows = [min(CHUNK, N - c * CHUNK) for c in range(NCH)]
    dims["CHUNK_ROWS"] = tuple(chunk_rows)
    return dict(dims=dims, cnt=cnt, gstart=gstart, n0=n0, n1=n1, nloc1=nloc1,
                dsthi1=dsthi1, dstlo1=dstlo1, ew1=ew1,
                dstcol2=dstcol2, ew2=ew2, idx2=idx2, mask2=mask2, cntinv=cntinv,
                x_slices=x_slices)


# ---------------------------------------------------------------- pass 1
def build_pass1(dims):
    NT1, WIN1, K1, NLOC1 = dims["NT1"], dims["WIN1"], dims["K1"], dims["NLOC1"]
    NB1 = NLOC1 // P  # x tiles per core

    nc = bacc.Bacc(None, target_bir_lowering=False)
    dsthi_d = nc.dram_tensor("dsthi", [P, NT1], f32, kind="ExternalInput")
    dstlo_d = nc.dram_tensor("dstlo", [P, NT1], f32, kind="ExternalInput")
    ew_d = nc.dram_tensor("ew", [P, NT1], f32, kind="ExternalInput")
    x_d = nc.dram_tensor("x", [NLOC1, FDIM], f32, kind="ExternalInput")
    dis_d = nc.dram_tensor("dis", [NLOC1], f32, kind="ExternalOutput")
    y_d = nc.dram_tensor("y", [NLOC1, FDIM], f32, kind="ExternalOutput")

    with tile.TileContext(nc) as tc:
        with (
            tc.tile_pool(name="one", bufs=1) as one,
            tc.tile_pool(name="st", bufs=8) as st,
            tc.tile_pool(name="xt", bufs=4) as xt,
            tc.tile_pool(name="ps", bufs=4, space="PSUM") as ps,
        ):
            dsthi_t = one.tile([P, NT1], f32)
            nc.sync.dma_start(out=dsthi_t[:], in_=dsthi_d[:, :])
            dstlo_t = one.tile([P, NT1], f32)
            nc.sync.dma_start(out=dstlo_t[:], in_=dstlo_d[:, :])
            ew_t = one.tile([P, NT1], f32)
            nc.sync.dma_start(out=ew_t[:], in_=ew_d[:, :])

            iota_i = one.tile([P, 16], i32)
            nc.gpsimd.iota(iota_i[:], [[1, 16]], channel_multiplier=0)
            iota_f = one.tile([P, 16], f32)
            nc.vector.tensor_copy(iota_f[:], iota_i[:])

            deg_grid = one.tile([16, WIN1 * 16], f32)
            for w in range(WIN1):
                acc = ps.tile([16, 16], f32, space="PSUM", tag="degacc")
                for k in range(K1):
                    t = w * K1 + k
                    shi = st.tile([P, 16], f32, tag="shi")
                    nc.vector.tensor_scalar(
                        out=shi[:], in0=iota_f[:],
                        scalar1=dsthi_t[:, t: t + 1], scalar2=None,
                        op0=Alu.is_equal,
                    )
                    slo = st.tile([P, 16], f32, tag="slo")
                    nc.vector.tensor_scalar(
                        out=slo[:], in0=iota_f[:],
                        scalar1=dstlo_t[:, t: t + 1], scalar2=ew_t[:, t: t + 1],
                        op0=Alu.is_equal, op1=Alu.mult,
                    )
                    nc.tensor.matmul(acc[:], shi[:], slo[:],
                                     start=(k == 0), stop=(k == K1 - 1))
                nc.vector.tensor_copy(deg_grid[:, w * 16:(w + 1) * 16], acc[:])

            # deg -> dis = 1/sqrt(max(deg,1))
            nc.vector.tensor_scalar(out=deg_grid[:], in0=deg_grid[:],
                                    scalar1=1.0, scalar2=None, op0=Alu.max)
            nc.scalar.activation(out=deg_grid[:], in_=deg_grid[:], func=Act.Sqrt)
            nc.vector.reciprocal(out=deg_grid[:], in_=deg_grid[:])
            # write dis: device col n = 256w + 16hi + lo; grid[hi, (w,lo)]
            src_ap = dis_d[:]
            dis_ap = bass.AP(tensor=src_ap.tensor, offset=0,
                             ap=[[16, 16], [256, WIN1], [1, 16]])
            nc.sync.dma_start(out=dis_ap,
                              in_=deg_grid[:].rearrange("hi (w lo) -> hi w lo", lo=16))

            # read back node-major: [p, c] = dis[c*128 + p]
            dis_nm = one.tile([P, NB1], f32)
            nc.sync.dma_start(out=dis_nm[:], in_=dis_d[:].rearrange("(c p) -> p c", p=P))

            for t in range(NB1):
                x_t = xt.tile([P, FDIM], f32, tag="x")
                nc.sync.dma_start(out=x_t[:], in_=x_d[t * P:(t + 1) * P, :])
                y_t = xt.tile([P, FDIM], f32, tag="y")
                nc.vector.tensor_scalar(out=y_t[:], in0=x_t[:],
                                        scalar1=dis_nm[:, t: t + 1], scalar2=None,
                                        op0=Alu.mult)
                nc.sync.dma_start(out=y_d[t * P:(t + 1) * P, :], in_=y_t[:])
    nc.compile()
    return nc


# ---------------------------------------------------------------- pass 2
def build_pass2(dims):
    NT2, WIN2, GSPAN, NLOC2 = dims["NT2"], dims["WIN2"], dims["GSPAN"], dims["NLOC2"]
    K2C, GPC, N = dims["K2C"], dims["GPC"], dims["N"]
    CHUNK_ROWS = dims["CHUNK_ROWS"]
    NCH = dims["NCH"]
    K2B = np.zeros(NCH + 1, np.int64)
    np.cumsum(np.asarray(K2C), out=K2B[1:])
    K2S = int(K2B[-1])
    NB2 = NLOC2 // P

    nc = bacc.Bacc(None, target_bir_lowering=False)
    y_d = nc.dram_tensor("y", [N, FDIM], f32, kind="ExternalInput")
    idx_d = nc.dram_tensor("idx", [P, NT2 * 8], i16, kind="ExternalInput")
    dstcol_d = nc.dram_tensor("dstcol", [P, NT2], f32, kind="ExternalInput")
    ew_d = nc.dram_tensor("ew", [P, NT2], f32, kind="ExternalInput")
    dis_d = nc.dram_tensor("dis", [1, NLOC2], f32, kind="ExternalInput")
    mask_d = nc.dram_tensor("mask", [1, NLOC2], f32, kind="ExternalInput")
    w_d = nc.dram_tensor("w", [FDIM, FDIM], f32, kind="ExternalInput")
    gn_d = nc.dram_tensor("gn", [P, 4], f32, kind="ExternalInput")  # b, gnw, gnb, gms
    cntinv_d = nc.dram_tensor("cntinv", [P, GPC], f32, kind="ExternalInput")
    hemb_d = nc.dram_tensor("hemb", [NLOC2, FDIM], f32, kind="ExternalOutput")
    flat_d = nc.dram_tensor("flat", [P, GPC], f32, kind="ExternalOutput")

    with tile.TileContext(nc) as tc:
        with (
            tc.tile_pool(name="one", bufs=1) as one,
            tc.tile_pool(name="gth", bufs=2) as gth,
            tc.tile_pool(name="ixp", bufs=3) as ixp,
            tc.tile_pool(name="st", bufs=8) as st,
            tc.tile_pool(name="wt", bufs=3) as wt,
            tc.tile_pool(name="gn", bufs=2) as gnp,
            tc.tile_pool(name="ps", bufs=2, space="PSUM") as ps,
            tc.tile_pool(name="ps2", bufs=2, space="PSUM") as ps2,
            tc.tile_pool(name="ps3", bufs=2, space="PSUM") as ps3,
        ):
            dstcol_t = one.tile([P, NT2], f32)
            nc.sync.dma_start(out=dstcol_t[:], in_=dstcol_d[:, :])
            ew_t = one.tile([P, NT2], f32)
            nc.sync.dma_start(out=ew_t[:], in_=ew_d[:, :])
            w_t = one.tile([FDIM, FDIM], f32r)
            nc.gpsimd.dma_start(out=w_t[:], in_=w_d[:, :].bitcast(f32r))
            gn_t = one.tile([P, 4], f32)
            nc.sync.dma_start(out=gn_t[:], in_=gn_d[:, :])
            cntinv_t = one.tile([P, GPC], f32)
            nc.sync.dma_start(out=cntinv_t[:], in_=cntinv_d[:, :])
            eps_t = one.tile([P, 1], f32)
            nc.vector.memset(eps_t[:], EPS)

            iota_i = one.tile([P, WINW], i32)
            nc.gpsimd.iota(iota_i[:], [[1, WINW]], channel_multiplier=0)
            iota_f = one.tile([P, WINW], f32)
            nc.vector.tensor_copy(iota_f[:], iota_i[:])

            hT = one.tile([P, NLOC2], f32)

            for w in range(WIN2):
                agg = ps.tile([P, WINW], f32, space="PSUM", tag="agg")
                idxw = ixp.tile([P, K2S * 8], i16, tag="idxw")
                nc.sync.dma_start(out=idxw[:], in_=idx_d[:, w * K2S * 8:(w + 1) * K2S * 8])
                first = True
                for ci in range(NCH):
                    kc = int(K2C[ci])
                    cb = ci * CHUNK
                    rows = CHUNK_ROWS[ci]
                    g_t = gth.tile([P, kc, FDIM], f32r, tag=f"g{ci}")
                    nc.gpsimd.dma_gather(
                        out_ap=g_t[:],
                        in_ap=y_d[cb: cb + rows, :].bitcast(f32r),
                        idxs_ap=idxw[:, int(K2B[ci]) * 8: int(K2B[ci]) * 8 + kc * 8],
                        num_idxs=kc * P,
                        num_idxs_reg=kc * P,
                        elem_size=FDIM,
                        single_packet=False,
                    )
                    for k in range(kc):
                        t = w * K2S + int(K2B[ci]) + k
                        s_t = st.tile([P, WINW], f32r, tag="s")
                        nc.vector.tensor_scalar(
                            out=s_t[:], in0=iota_f[:],
                            scalar1=dstcol_t[:, t: t + 1],
                            scalar2=ew_t[:, t: t + 1],
                            op0=Alu.is_equal, op1=Alu.mult,
                        )
                        nc.tensor.matmul(agg[:], g_t[:, k, :], s_t[:],
                                         start=first, stop=(ci == NCH - 1 and k == kc - 1))
                        first = False
                # dis[dst] fold
                dis_s = wt.tile([P, WINW], f32, tag="dis")
                nc.gpsimd.dma_start(
                    out=dis_s[:],
                    in_=dis_d[0:1, w * WINW:(w + 1) * WINW].to_broadcast([P, WINW]),
                )
                aggw = wt.tile([P, WINW], f32r, tag="aggw")
                nc.vector.tensor_tensor(out=aggw[:], in0=agg[:], in1=dis_s[:], op=Alu.mult)
                # h.T window = W.T @ aggw
                h_ps = ps2.tile([P, WINW], f32, space="PSUM", tag="h")
                nc.tensor.matmul(h_ps[:], w_t[:], aggw[:], start=True, stop=True)
                nc.scalar.copy(hT[:, w * WINW:(w + 1) * WINW], h_ps[:])

            # ---------------- GraphNorm + ReLU + masked max ----------------
            b_c = gn_t[:, 0:1]
            gnw_c = gn_t[:, 1:2]
            gnb_c = gn_t[:, 2:3]
            gms_c = gn_t[:, 3:4]
            flat_t = one.tile([P, GPC], f32)
            for s in range(GPC):
                blk = hT[:, s * GSPAN:(s + 1) * GSPAN]
                s1 = gnp.tile([P, 1], f32, tag="s1")
                sc1 = gnp.tile([P, GSPAN], f32, tag="scr")
                nc.scalar.activation(out=sc1[:], in_=blk, func=Act.Copy, accum_out=s1[:])
                s2 = gnp.tile([P, 1], f32, tag="s2")
                sc2 = gnp.tile([P, GSPAN], f32, tag="scr")
                nc.scalar.activation(out=sc2[:], in_=blk, func=Act.Square, accum_out=s2[:])

                civ = cntinv_t[:, s: s + 1]
                mean0 = gnp.tile([P, 1], f32, tag="m0")
                nc.vector.tensor_tensor(out=mean0[:], in0=s1[:], in1=civ, op=Alu.mult)
                ms = gnp.tile([P, 1], f32, tag="ms")
                nc.vector.tensor_tensor(out=ms[:], in0=mean0[:], in1=b_c, op=Alu.add)
                msg = gnp.tile([P, 1], f32, tag="msg")
                nc.vector.tensor_tensor(out=msg[:], in0=ms[:], in1=gms_c, op=Alu.mult)
                a_t = gnp.tile([P, 1], f32, tag="a")
                nc.vector.tensor_tensor(out=a_t[:], in0=b_c, in1=msg[:], op=Alu.subtract)
                # var = S2/cnt + 2*A*mean0 + A^2
                v1 = gnp.tile([P, 1], f32, tag="v1")
                nc.vector.tensor_tensor(out=v1[:], in0=s2[:], in1=civ, op=Alu.mult)
                v2 = gnp.tile([P, 1], f32, tag="v2")
                nc.vector.tensor_tensor(out=v2[:], in0=a_t[:], in1=mean0[:], op=Alu.mult)
                v3 = gnp.tile([P, 1], f32, tag="v3")
                nc.vector.tensor_scalar(out=v3[:], in0=v2[:], scalar1=2.0, scalar2=None,
                                        op0=Alu.mult)
                v4 = gnp.tile([P, 1], f32, tag="v4")
                nc.vector.tensor_tensor(out=v4[:], in0=a_t[:], in1=a_t[:], op=Alu.mult)
                v5 = gnp.tile([P, 1], f32, tag="v5")
                nc.vector.tensor_tensor(out=v5[:], in0=v1[:], in1=v3[:], op=Alu.add)
                var = gnp.tile([P, 1], f32, tag="var")
                nc.vector.tensor_tensor(out=var[:], in0=v5[:], in1=v4[:], op=Alu.add)
                # std = sqrt(var + eps); k = gnw/std
                std = gnp.tile([P, 1], f32, tag="std")
                nc.scalar.activation(out=std[:], in_=var[:], func=Act.Sqrt, bias=eps_t[:, 0:1])
                nc.vector.reciprocal(out=std[:], in_=std[:])
                k_t = gnp.tile([P, 1], f32, tag="k")
                nc.vector.tensor_tensor(out=k_t[:], in0=std[:], in1=gnw_c, op=Alu.mult)
                ca1 = gnp.tile([P, 1], f32, tag="ca1")
                nc.vector.tensor_tensor(out=ca1[:], in0=a_t[:], in1=k_t[:], op=Alu.mult)
                ca = gnp.tile([P, 1], f32, tag="ca")
                nc.vector.tensor_tensor(out=ca[:], in0=ca1[:], in1=gnb_c, op=Alu.add)
                # h_emb = relu(h0*k + ca), then mask, then max
                nc.scalar.activation(out=blk, in_=blk, func=Act.Relu,
                                     scale=k_t[:, 0:1], bias=ca[:, 0:1])
                msk = wt.tile([P, GSPAN], f32, tag="msk")
                nc.gpsimd.dma_start(
                    out=msk[:],
                    in_=mask_d[0:1, s * GSPAN:(s + 1) * GSPAN].to_broadcast([P, GSPAN]),
                )
                nc.vector.tensor_tensor(out=blk, in0=blk, in1=msk[:], op=Alu.mult)
                nc.vector.tensor_reduce(out=flat_t[:, s: s + 1], in_=blk,
                                        axis=mybir.AxisListType.X, op=Alu.max)
            nc.sync.dma_start(out=flat_d[:, :], in_=flat_t[:])

            # ---------------- transpose h_emb out ----------------
            ident = one.tile([P, P], f32)
            make_identity(nc, ident[:])
            for t in range(NB2):
                tr_ps = ps3.tile([P, P], f32, space="PSUM", tag="tr")
                nc.tensor.transpose(out=tr_ps[:], in_=hT[:, t * P:(t + 1) * P],
                                    identity=ident[:])
                tr_sb = wt.tile([P, P], f32, tag="tr_sb")
                nc.scalar.copy(tr_sb[:], tr_ps[:])
                nc.sync.dma_start(out=hemb_d[t * P:(t + 1) * P, :], in_=tr_sb[:])
    nc.compile()
    return nc


# ---------------------------------------------------------------- driver
_NEFF_CACHE = {}


def _sim_exec(nc, in_maps):
    """CoreSim execution, one core at a time. Returns list of output dicts."""
    from concourse.bass_interp import CoreSim
    out_names = []
    for alloc in nc.m.functions[0].allocations:
        if isinstance(alloc, mybir.MemoryLocationSet) and alloc.kind == "ExternalOutput":
            out_names.append(alloc.memorylocations[0].name)
    results = []
    for im in in_maps:
        sim = CoreSim(nc, trace=False)
        for k, v in im.items():
            sim.tensor(k)[:] = v
        sim.simulate()
        results.append({k: np.array(sim.tensor(k)) for k in out_names})
    return results


def run(inputs, edge_index, batch, edge_weight, W, b,
        gn_weight, gn_bias, gn_mean_scale, use_hw=True):
    inputs = np.ascontiguousarray(inputs, dtype=np.float32)
    edge_index = np.ascontiguousarray(edge_index)
    batch_np = np.ascontiguousarray(batch)
    edge_weight = np.ascontiguousarray(edge_weight, dtype=np.float32)

    pl = plan(inputs, edge_index, batch_np, edge_weight)
    dims = pl["dims"]
    key = tuple(sorted((k, v) for k, v in dims.items()))
    if key not in _NEFF_CACHE:
        _NEFF_CACHE[key] = (build_pass1(dims), build_pass2(dims))
    nc1, nc2 = _NEFF_CACHE[key]

    # ---- pass 1 ----
    in1 = [dict(dsthi=pl["dsthi1"][c], dstlo=pl["dstlo1"][c], ew=pl["ew1"][c],
                x=pl["x_slices"][c]) for c in range(NCORES)]
    if use_hw:
        res1 = run_bass_kernel_spmd(nc1, in1, core_ids=list(range(NCORES))).results
    else:
        res1 = _sim_exec(nc1, in1)

    # ---- host glue ----
    N, G = dims["N"], dims["G"]
    GPC, GSPAN, NLOC2 = dims["GPC"], dims["GSPAN"], dims["NLOC2"]
    cnt, gstart, n0 = pl["cnt"], pl["gstart"], pl["n0"]
    Y = np.zeros((N, FDIM), np.float32)
    dis2 = np.zeros((NCORES, 1, NLOC2), np.float32)
    for c in range(NCORES):
        nl = pl["nloc1"][c]
        Y[n0[c]: n0[c] + nl] = res1[c]["y"][:nl]
        dis_raw = res1[c]["dis"]
        for s in range(GPC):
            g = c * GPC + s
            lo = gstart[g] - n0[c]
            dis2[c, 0, s * GSPAN: s * GSPAN + cnt[g]] = dis_raw[lo: lo + cnt[g]]

    gn = np.stack([np.broadcast_to(v.astype(np.float32), (P,)) for v in
                   (b, gn_weight, gn_bias, gn_mean_scale)], axis=1)
    in2 = [dict(y=Y, idx=pl["idx2"][c], dstcol=pl["dstcol2"][c], ew=pl["ew2"][c],
                dis=dis2[c], mask=pl["mask2"][c],
                w=np.ascontiguousarray(W, np.float32), gn=gn,
                cntinv=pl["cntinv"][c]) for c in range(NCORES)]
    if use_hw:
        res2 = run_bass_kernel_spmd(nc2, in2, core_ids=list(range(NCORES))).results
    else:
        res2 = _sim_exec(nc2, in2)

    # ---- assemble ----
    h_emb = np.zeros((N, FDIM), np.float32)
    flat = np.full((G, FDIM), -np.inf, np.float32)
    for c in range(NCORES):
        he = res2[c]["hemb"]
        fl = res2[c]["flat"]  # [128 F, GPC]
        for s in range(GPC):
            g = c * GPC + s
            if cnt[g] > 0:
                h_emb[gstart[g]: gstart[g + 1]] = he[s * GSPAN: s * GSPAN + cnt[g]]
                flat[g] = fl[:, s]
    return h_emb, flat


_PASS_TIMES = {}


def kernel(inputs, edge_index, batch, edge_weight, W, b,
           gn_weight, gn_bias, gn_mean_scale):
    """Full GCN block. Returns (h_emb, flat, edge_index, edge_weight, batch)."""
    h_emb, flat = run(np.asarray(inputs), np.asarray(edge_index),
                      np.asarray(batch), np.asarray(edge_weight),
                      np.asarray(W), np.asarray(b), np.asarray(gn_weight),
                      np.asarray(gn_bias), np.asarray(gn_mean_scale),
                      use_hw=True)
    return (h_emb, flat, edge_index, edge_weight, batch)
